# revision 1
# baseline (speedup 1.0000x reference)
"""Trainium2 kernel for nn_Direction: out = input @ qr(weight + 1e-8).Q.T

input: [524288, 20] f32, weight: [512, 20] f32 -> out: [524288, 512] f32.

Strategy (data-parallel across 8 NeuronCores, batch-sharded):
  - QR of the tiny 512x20 weight on host; Q is replicated to every core.
  - input and Q are split into bf16 hi/lo pairs on host so the PE runs at
    full bf16 rate (fp32 matmul is quarter rate). out = x_hi@Q_hi.T +
    x_lo@Q_hi.T + x_hi@Q_lo.T as ONE K=60 matmul per 128-row tile
    (rel err ~1e-5; the dropped x_lo@Q_lo term is ~2^-18).
  - input is pre-transposed on host to [60, B] so the contraction dim is
    the partition dim -- no on-chip transpose.
  - per tile: matmul -> PSUM [128,512] -> DVE/ACT copy -> SBUF staging ->
    2MB DMAs to HBM. The 1GB output write is the roofline (~390us/core).
"""

from contextlib import ExitStack

import ml_dtypes
import numpy as np

BATCH, MDIM, ODIM = 524288, 20, 512
NCORES = 8
BC = BATCH // NCORES  # 65536 rows per core
KSTACK = 3 * MDIM  # 60: [x_hi; x_lo; x_hi] rows

_BF16 = ml_dtypes.bfloat16


def build_bass(Bc: int, chunk: int, G: int):
    """Build the per-core Bass program. Returns compiled nc.

    Bc: batch rows per core; chunk: batch columns per input DMA;
    G: number of [128,512] tiles per output staging buffer / out-DMA.
    """
    import concourse.bacc as bacc
    import concourse.mybir as mybir
    import concourse.tile as tile

    assert Bc % chunk == 0 and chunk % (G * 128) == 0

    bf16 = mybir.dt.bfloat16
    f32 = mybir.dt.float32

    nc = bacc.Bacc(
        "TRN2",
        target_bir_lowering=False,
        debug=False,
        enable_asserts=False,
        num_devices=NCORES,
    )

    xT = nc.dram_tensor("xT", [KSTACK, Bc], bf16, kind="ExternalInput").ap()
    q3 = nc.dram_tensor("q3", [KSTACK, ODIM], bf16, kind="ExternalInput").ap()
    out = nc.dram_tensor("out", [Bc, ODIM], f32, kind="ExternalOutput").ap()

    n_stages = Bc // (G * 128)
    stages_per_chunk = chunk // (G * 128)
    # out rows viewed as [stage, tile-in-stage, partition]
    out_v = out.rearrange("(s t p) n -> s p t n", t=G, p=128)

    with tile.TileContext(nc) as tc, ExitStack() as ctx:
        qp = ctx.enter_context(tc.tile_pool(name="q", bufs=1))
        inp = ctx.enter_context(tc.tile_pool(name="inp", bufs=3))
        outp = ctx.enter_context(tc.tile_pool(name="outp", bufs=3))
        psp = ctx.enter_context(tc.tile_pool(name="ps", bufs=8, space="PSUM"))

        q3t = qp.tile([KSTACK, ODIM], bf16)
        nc.sync.dma_start(out=q3t[:], in_=q3[:])

        gidx = 0
        for ci in range(Bc // chunk):
            it = inp.tile([KSTACK, chunk], bf16)
            nc.sync.dma_start(out=it[:], in_=xT[:, ci * chunk : (ci + 1) * chunk])
            for s in range(stages_per_chunk):
                st = outp.tile([128, G, ODIM], f32)
                for t in range(G):
                    col = s * G * 128 + t * 128
                    ps = psp.tile([128, ODIM], f32)
                    nc.tensor.matmul(
                        ps[:], it[:, col : col + 128], q3t[:], start=True, stop=True
                    )
                    if gidx % 2 == 0:
                        nc.vector.tensor_copy(st[:, t, :], ps[:])
                    else:
                        nc.scalar.copy(st[:, t, :], ps[:])
                    gidx += 1
                nc.sync.dma_start(
                    out=out_v[ci * stages_per_chunk + s], in_=st[:]
                )
    assert gidx == Bc // 128 and (ci + 1) * stages_per_chunk == n_stages
    nc.compile()
    return nc


def prepare_inputs(input: np.ndarray, weight: np.ndarray):
    """Host-side marshalling: QR, bf16 hi/lo split, transpose, shard."""
    x = np.ascontiguousarray(input, dtype=np.float32)
    w = np.ascontiguousarray(weight, dtype=np.float32)

    Q, _ = np.linalg.qr(w + np.float32(1e-8), mode="reduced")  # [512, 20] f32
    Q = Q.astype(np.float32)
    Q_hi = Q.astype(_BF16)
    Q_lo = (Q - Q_hi.astype(np.float32)).astype(_BF16)

    x_hi = x.astype(_BF16)
    x_lo = (x - x_hi.astype(np.float32)).astype(_BF16)

    # stacked transposed input rows: [x_hi; x_lo; x_hi]
    stacked = np.empty((KSTACK, BATCH), dtype=_BF16)
    stacked[0:MDIM] = x_hi.T
    stacked[MDIM : 2 * MDIM] = x_lo.T
    stacked[2 * MDIM :] = x_hi.T

    # rhs rows: [Q_hi.T; Q_hi.T; Q_lo.T] -> pairs (x_hi,Q_hi),(x_lo,Q_hi),(x_hi,Q_lo)
    q3 = np.empty((KSTACK, ODIM), dtype=_BF16)
    q3[0:MDIM] = Q_hi.T
    q3[MDIM : 2 * MDIM] = Q_hi.T
    q3[2 * MDIM :] = Q_lo.T

    in_maps = [
        {
            "xT": np.ascontiguousarray(stacked[:, c * BC : (c + 1) * BC]),
            "q3": q3,
        }
        for c in range(NCORES)
    ]
    return in_maps


_CACHE = {}


def _compiled(Bc, chunk, G):
    key = (Bc, chunk, G)
    if key not in _CACHE:
        _CACHE[key] = build_bass(Bc, chunk, G)
    return _CACHE[key]


def kernel(input: np.ndarray, weight: np.ndarray) -> np.ndarray:
    from concourse.bass_utils import run_bass_kernel_spmd

    assert input.shape == (BATCH, MDIM) and weight.shape == (ODIM, MDIM)
    nc = _compiled(BC, 8192, 8)
    in_maps = prepare_inputs(input, weight)
    res = run_bass_kernel_spmd(nc, in_maps, list(range(NCORES)))
    out = np.concatenate([r["out"] for r in res.results], axis=0)
    return np.ascontiguousarray(out, dtype=np.float32)
